# revision 3
# baseline (speedup 1.0000x reference)
"""Trainium2 Bass kernel v2: fp8-DoubleRow QK projection + block-diag attention.

Problem shapes (hardcoded from the task spec):
  x:        (2, 1024, 64, 512) fp32
  w_qkv:    (1536, 512) fp32   -> q|k|v each 512 feats = 8 heads x 64
  pos_bias: (8, 64, 64) fp32
  focus_present_mask: (2,) bool

Key design points vs v1 (313 us):
  - QK projection runs in fp8e4 DoubleRow mode (0.5 cyc/row): q,k errors are
    absorbed by the softmax (logits dominated by the exact pos_bias), and
    weights are pre-scaled by 128 (power of 2) to avoid fp8 subnormals; the
    2^-17 compensation folds into the exp activation's scale for free.
    Simulated end-to-end rel_fro = 3.6e-3 (bf16 baseline 2.7e-3).
  - V projections and PV stay bf16 (V feeds the output directly; fp8 there
    measured 3.7e-2 > 2e-2 tolerance).
  - sim matmuls use block-diagonal k (kbd) packing: one [128,128]x[128,64]
    matmul covers two positions (PE cost is rows-only, so 2x fewer rows).
  - Work is organized in blocks of 1024 tokens (16 positions, 8 pairs);
    fp8 QK weights stay stationary across 4 matmuls of 256 tokens each.
  - QK PSUM: [64, 8,2,64] tiles (DoubleRow out is fixed at partitions 0:63;
    the compiler rejects tile_position=(0,64) in DR mode), ring of 2 (4
    banks). V-projection PSUM is a single bank: V units are emitted one per
    attention pair, so the reuse window (~2.5us) dwarfs the drain time.
  - PE p-state warm-up: a train of dummy matmuls issued under the initial
    DMA so real work starts at 2.4 GHz.
  - All outputs stored bf16, cast to fp32 on host.
"""

import numpy as np

import concourse.bass as bass
import concourse.bacc as bacc
import concourse.mybir as mybir
import concourse.tile as tile
from concourse.bass_utils import run_bass_kernel_spmd

HEADS = 8
DH = 64
NTOK = 64          # tokens per spatial position
DIM = 512
N_CORES = 8
P = 128
BLK = 1024         # tokens per block = 16 positions = 8 pairs
F32 = mybir.dt.float32
BF16 = mybir.dt.bfloat16
F8 = mybir.dt.float8e4

W_SCALE = 128.0            # fp8 weight pre-scale (power of 2)
EXP_SCALE = 1.0 / (W_SCALE * W_SCALE * 8.0)   # undo w scales + softmax 1/sqrt(64)

LAST_RESULT = None
_KERNEL_CACHE: dict = {}


def _ensure_ntff_hook():
    """Make BASS_TRACE=1 usable: bass_utils' axon trace path imports
    antenv.axon_hooks, which some images lack."""
    import sys
    import types

    try:
        import antenv.axon_hooks  # noqa: F401

        return
    except ImportError:
        pass
    try:
        import antenv
        from trn_agent_boot.trn_boot import _ntff_profile_via_ctypes
    except ImportError:
        return
    mod = types.ModuleType("antenv.axon_hooks")
    _state = {"hook": None}
    mod.set_axon_ntff_profile_hook = lambda h: _state.__setitem__("hook", h)
    mod.get_axon_ntff_profile_hook = lambda: _state["hook"]
    sys.modules["antenv.axon_hooks"] = mod
    antenv.axon_hooks = mod
    import os as _os

    so = "/opt/axon/libaxon_pjrt.so"
    if _os.path.exists(so):
        try:
            mod.set_axon_ntff_profile_hook(_ntff_profile_via_ctypes(so))
        except Exception:
            pass


class _Sched:
    """Emission helper holding tiles + engine rotation state."""

    def __init__(self, nc):
        self.nc = nc
        self.drain_rr = 0

    def drain_engine(self):
        # PSUM drains: only Scalar and Vector may read PSUM (GpSimd cannot)
        engines = ("scalar", "vector")
        e = engines[self.drain_rr % len(engines)]
        self.drain_rr += 1
        return e

    def copy(self, out, in_, engine):
        nc = self.nc
        if engine == "scalar":
            nc.scalar.copy(out=out, in_=in_)
        elif engine == "vector":
            nc.vector.tensor_copy(out=out, in_=in_)
        else:
            nc.gpsimd.tensor_copy(out=out, in_=in_)


def _build_kernel(a_tok: int, v_tok: int):
    """a_tok, v_tok: tokens per core needing attention / V-only (mult of BLK)."""
    nc = bacc.Bacc("TRN2")
    EXP = mybir.ActivationFunctionType.Exp
    DR = mybir.MatmulPerfMode.DoubleRow

    nb_a = a_tok // BLK
    nb_v = v_tok // BLK

    wqk8 = nc.dram_tensor("wqk8", [DIM, 2 * DIM], F8, kind="ExternalInput")
    wvT = nc.dram_tensor("wvT", [DIM, DIM], BF16, kind="ExternalInput")
    ebiasT = nc.dram_tensor("ebiasT", [P, DIM], BF16, kind="ExternalInput")
    xa8T = xaT = out_a = None
    if a_tok:
        xa8T = nc.dram_tensor("xa8T", [DIM, a_tok], F8, kind="ExternalInput")
        xaT = nc.dram_tensor("xaT", [DIM, a_tok], BF16, kind="ExternalInput")
        out_a = nc.dram_tensor("out_a", [a_tok, DIM], BF16, kind="ExternalOutput")
    xvT = out_v = None
    if v_tok:
        xvT = nc.dram_tensor("xvT", [DIM, v_tok], BF16, kind="ExternalInput")
        out_v = nc.dram_tensor("out_v", [v_tok, DIM], BF16, kind="ExternalOutput")

    with tile.TileContext(nc) as tc:
        with (
            tc.tile_pool(name="const", bufs=1) as const,
            tc.tile_pool(name="x8", bufs=2) as x8pool,
            tc.tile_pool(name="x16", bufs=2) as x16pool,
            tc.tile_pool(name="xv", bufs=2) as xvpool,
            tc.tile_pool(name="qst", bufs=2) as qstpool,
            tc.tile_pool(name="kbd", bufs=1) as kbdpool,
            tc.tile_pool(name="vt", bufs=1) as vtpool,
            tc.tile_pool(name="ee", bufs=2) as epool,
            tc.tile_pool(name="ot", bufs=2) as otpool,
            tc.tile_pool(name="ov", bufs=2) as ovpool,
            tc.tile_pool(name="rr", bufs=4) as rpool,
            tc.tile_pool(name="pq", bufs=2, space="PSUM") as pp_qk,
            tc.tile_pool(name="pv", bufs=1, space="PSUM") as pp_proj,
            tc.tile_pool(name="psx", bufs=1, space="PSUM") as pp_s,
            tc.tile_pool(name="pox", bufs=1, space="PSUM") as pp_o,
        ):
            sched = _Sched(nc)

            # ---- persistent SBUF tiles
            warm_sb = const.tile([P, DH], BF16)
            wv_sb = const.tile([P, 4, DIM], BF16)
            wqk8_sb = const.tile([P, 2, 2, 2 * DIM], F8)
            ebias_sb = const.tile([P, DIM], BF16)

            # kbd: block-diag k, 2 ring slots; off-diag quadrants must be 0.
            kbd = [
                kbdpool.tile([P, HEADS, 8, P], BF16, tag=f"kbd{s}", name=f"kbd{s}")
                for s in range(2)
            ]
            # vt: 16 persistent slots (2 blocks x 8 pairs); ones column written
            # once and reused (drains only touch cols 0:64)
            vts_all = [
                vtpool.tile([P, HEADS, 65], BF16, tag=f"vt{i}", name=f"vt{i}")
                for i in range(16)
            ]
            for i, vt in enumerate(vts_all):
                if i % 2 == 0:
                    nc.gpsimd.memset(vt[:, :, 64:65], 1.0)
                else:
                    nc.vector.memset(vt[:, :, 64:65], 1.0)
            # zero the off-diagonal quadrants once (split across engines)
            nc.vector.memset(kbd[0][0:64, :, :, 64:128], 0.0)
            nc.gpsimd.memset(kbd[0][64:128, :, :, 0:64], 0.0)
            nc.scalar.memzero(kbd[1][0:64, :, :, 64:128])
            nc.gpsimd.memset(kbd[1][64:128, :, :, 0:64], 0.0)

            # ---- PE warm-up: dummy matmul train during initial DMA
            # (shares the sim PSUM slot via same pool+tag; finishes long
            # before the first sim matmul)
            nc.vector.memset(warm_sb[:], 0.0)
            pwarm = pp_s.tile([DH, DH], F32, tag="ps_s", name="pwarm")
            for _ in range(56):
                nc.tensor.matmul(
                    pwarm[:],
                    lhsT=warm_sb[:, 0:64],
                    rhs=warm_sb[:],
                    start=True,
                    stop=True,
                )

            # ---- input DMAs for block 0 + constants
            xa8_r = xa8T[:, :].rearrange("(kp i p) t -> p kp i t", kp=2, i=2) if a_tok else None
            xa16_r = xaT[:, :].rearrange("(k p) t -> p k t", p=P) if a_tok else None
            xv_r = xvT[:, :].rearrange("(k p) t -> p k t", p=P) if v_tok else None

            x8_t = x16_t = None
            if nb_a:
                x8_t = x8pool.tile([P, 2, 2, BLK], F8, tag="x8")
                nc.sync.dma_start(x8_t[:], xa8_r[:, :, :, 0:BLK])
            nc.sync.dma_start(
                wqk8_sb[:], wqk8[:, :].rearrange("(kp i p) e -> p kp i e", kp=2, i=2)
            )
            if nb_a:
                x16_t = x16pool.tile([P, 4, BLK], BF16, tag="x16")
                nc.sync.dma_start(x16_t[:], xa16_r[:, :, 0:BLK])
            wvT_r = wvT[:, :].rearrange("(k p) e -> p k e", p=P)
            for kt in range(4):
                nc.sync.dma_start(wv_sb[:, kt], wvT_r[:, kt])
            nc.sync.dma_start(ebias_sb[:], ebiasT[:, :])
            xv_t = None
            if nb_v:
                xv_t = xvpool.tile([P, 4, BLK], BF16, tag="xv")
                nc.sync.dma_start(xv_t[:], xv_r[:, :, 0:BLK])

            def qk_fb(b, fb, x8_tile, qst_t, kbd_t):
                """One feature-block (64 feats) of the fp8 DR QK projection.

                Each 512-token group is exactly one PSUM bank and is written by
                ONE matmul per kp (1024-free moving): a start=True write marks
                its whole 2KB bank pending-zero, so accumulation groups must
                own a full bank. kp-outer keeps LDWEIGHTS at 2 per fb."""
                rows = pp_qk.tile([DH, 8, 2, DH], F32, tag="pq", name="pq")
                for kp in range(2):
                    for g2 in range(2):
                        nc.tensor.matmul(
                            rows[:, g2 * 4 : (g2 + 1) * 4],
                            lhsT=wqk8_sb[:, kp, :, fb * 64 : (fb + 1) * 64],
                            rhs=x8_tile[:, kp, :, g2 * 512 : (g2 + 1) * 512],
                            start=(kp == 0),
                            stop=(kp == 1),
                            perf_mode=DR,
                            skip_group_check=True,
                        )
                # drains: A-positions -> partitions 0:64, B -> 64:128
                if fb < 8:
                    h = fb
                    sched.copy(qst_t[0:64, h], rows[:, :, 0, :], sched.drain_engine())
                    sched.copy(qst_t[64:128, h], rows[:, :, 1, :], sched.drain_engine())
                else:
                    h = fb - 8
                    sched.copy(
                        kbd_t[0:64, h, :, 0:64], rows[:, :, 0, :], sched.drain_engine()
                    )
                    sched.copy(
                        kbd_t[64:128, h, :, 64:128],
                        rows[:, :, 1, :],
                        sched.drain_engine(),
                    )

            def vattn_tt(b, tt, x16_tile):
                """V projection for one token-pair (128 tokens) of an attn block."""
                psv = pp_proj.tile([P, DIM], F32, tag="psv")
                for kt in range(4):
                    nc.tensor.matmul(
                        psv[:],
                        lhsT=x16_tile[:, kt, tt * 128 : (tt + 1) * 128],
                        rhs=wv_sb[:, kt, :],
                        start=(kt == 0),
                        stop=(kt == 3),
                    )
                vt = vts_all[(b % 2) * 8 + tt]
                eng = "vector" if tt % 2 == 0 else "scalar"
                sched.copy(
                    vt[:, :, 0:64], psv[:].rearrange("p (h d) -> p h d", h=HEADS), eng
                )
                return vt

            def vonly_tt(vb, tt, xv_tile, ov_t):
                psv = pp_proj.tile([P, DIM], F32, tag="psv")
                for kt in range(4):
                    nc.tensor.matmul(
                        psv[:],
                        lhsT=xv_tile[:, kt, tt * 128 : (tt + 1) * 128],
                        rhs=wv_sb[:, kt, :],
                        start=(kt == 0),
                        stop=(kt == 3),
                    )
                eng = "scalar" if tt % 2 == 0 else "vector"
                sched.copy(ov_t[:, tt, :], psv[:], eng)
                nc.sync.dma_start(
                    out_v[vb * BLK + tt * 128 : vb * BLK + (tt + 1) * 128, :],
                    ov_t[:, tt, :],
                )

            def attention_pair(b, g, qst_t, kbd_t, vts, ot_t):
                pss = pp_s.tile([P, DIM], F32, tag="ps_s")
                for h in range(HEADS):
                    nc.tensor.matmul(
                        pss[:, h * 64 : (h + 1) * 64],
                        lhsT=kbd_t[:, h, g, :],
                        rhs=qst_t[:, h, g, :],
                        start=True,
                        stop=True,
                    )
                e_raw = epool.tile([P, DIM], BF16, tag="Eraw")
                nc.scalar.activation(e_raw[:], pss[:], EXP, scale=EXP_SCALE)
                e_t = epool.tile([P, DIM], BF16, tag="E")
                # all-SBUF elementwise -> GpSimd (frees DVE for PSUM drains)
                nc.gpsimd.tensor_tensor(
                    e_t[:], e_raw[:], ebias_sb[:], mybir.AluOpType.mult
                )
                pvt = pp_o.tile([P, 2, 512], F32, tag="pvt")
                vt = vts[g]
                for h in range(HEADS):
                    hb, hh = h // 4, h % 4
                    for ab in range(2):
                        nc.tensor.matmul(
                            pvt[hb * 64 : (hb + 1) * 64, ab, hh * 65 : hh * 65 + 65],
                            lhsT=e_t[ab * 64 : (ab + 1) * 64, h * 64 : (h + 1) * 64],
                            rhs=vt[ab * 64 : (ab + 1) * 64, h, :],
                            start=True,
                            stop=True,
                            tile_position=(ab * 64, hb * 64),
                        )
                pvt_r = pvt[:, :, 0:260].rearrange("p ab (h x) -> p ab h x", h=4)
                rec = rpool.tile([P, 2, 4, 1], F32, tag="rec")
                nc.vector.reciprocal(rec[:], pvt_r[:, :, :, 64:65])
                nc.vector.tensor_tensor(
                    ot_t[:, g].rearrange("p (ab h d) -> p ab h d", ab=2, h=4),
                    pvt_r[:, :, :, 0:64],
                    rec[:].to_broadcast((P, 2, 4, 64)),
                    mybir.AluOpType.mult,
                )

            def store_attn_block(b, ot_t):
                row0 = b * BLK
                for hb in range(2):
                    src = ot_t[hb * 64 : (hb + 1) * 64, :, :].rearrange(
                        "t g (ab cc) -> t g ab cc", ab=2
                    )
                    dst = out_a[
                        row0 : row0 + BLK, hb * 256 : (hb + 1) * 256
                    ].rearrange("(g ab t) cc -> t g ab cc", g=8, ab=2)
                    nc.sync.dma_start(dst, src)

            # ================= emission =================
            # filler generator: projection work to interleave between pairs
            vonly_units = [(vb, tt) for vb in range(nb_v) for tt in range(8)]
            vidx = 0
            ov_t = None
            cur_xv = xv_t

            def emit_vonly():
                nonlocal vidx, ov_t, cur_xv
                if vidx >= len(vonly_units):
                    return False
                vb, tt = vonly_units[vidx]
                if tt == 0:
                    ov_t = ovpool.tile([P, 8, DIM], BF16, tag="ov", name="ov")
                vonly_tt(vb, tt, cur_xv, ov_t)
                vidx += 1
                # prefetch next vonly block's x and rotate tile
                if tt == 7 and vb + 1 < nb_v:
                    cur_xv = xvpool.tile([P, 4, BLK], BF16, tag="xv", name="xv")
                    nc.sync.dma_start(
                        cur_xv[:], xv_r[:, :, (vb + 1) * BLK : (vb + 2) * BLK]
                    )
                return True

            # --- prologue: QK(0) + Vattn(0), no attention yet
            qst = [None, None]
            vts_ring = [[None] * 8, [None] * 8]
            if nb_a:
                qst[0] = qstpool.tile([P, HEADS, 8, DH], BF16, tag="qst", name="qst")
                for fb in range(16):
                    qk_fb(0, fb, x8_t, qst[0], kbd[0])
                    if fb % 2 == 1:
                        tt = fb // 2
                        vts_ring[0][tt] = vattn_tt(0, tt, x16_t)
                # prefetch block 1 inputs
                if nb_a > 1:
                    x8_t = x8pool.tile([P, 2, 2, BLK], F8, tag="x8", name="x8")
                    nc.sync.dma_start(x8_t[:], xa8_r[:, :, :, BLK : 2 * BLK])
                    x16_t = x16pool.tile([P, 4, BLK], BF16, tag="x16", name="x16")
                    nc.sync.dma_start(x16_t[:], xa16_r[:, :, BLK : 2 * BLK])

                for b in range(nb_a):
                    s = b % 2
                    ns = (b + 1) % 2
                    ot_t = otpool.tile([P, 8, DIM], BF16, tag="ot", name="ot")
                    if b + 1 < nb_a:
                        qst[ns] = qstpool.tile(
                            [P, HEADS, 8, DH], BF16, tag="qst", name="qst"
                        )
                    for g in range(8):
                        attention_pair(b, g, qst[s], kbd[s], vts_ring[s], ot_t)
                        # filler: 2 QK fbs + 1 Vattn tt of block b+1, 1 V-only tt
                        if b + 1 < nb_a:
                            qk_fb(b + 1, 2 * g, x8_t, qst[ns], kbd[ns])
                            emit_vonly()
                            qk_fb(b + 1, 2 * g + 1, x8_t, qst[ns], kbd[ns])
                            vts_ring[ns][g] = vattn_tt(b + 1, g, x16_t)
                        else:
                            emit_vonly()
                            emit_vonly()
                    store_attn_block(b, ot_t)
                    # prefetch block b+2 inputs
                    if b + 2 < nb_a:
                        x8_t = x8pool.tile([P, 2, 2, BLK], F8, tag="x8", name="x8")
                        nc.sync.dma_start(
                            x8_t[:], xa8_r[:, :, :, (b + 2) * BLK : (b + 3) * BLK]
                        )
                        x16_t = x16pool.tile([P, 4, BLK], BF16, tag="x16", name="x16")
                        nc.sync.dma_start(
                            x16_t[:], xa16_r[:, :, (b + 2) * BLK : (b + 3) * BLK]
                        )
            # drain any remaining V-only work
            while emit_vonly():
                pass

    nc.finalize()
    return nc


def _pad_positions(idx: np.ndarray, mult: int) -> np.ndarray:
    if len(idx) % mult == 0:
        return idx
    pad = mult - len(idx) % mult
    return np.concatenate([idx, np.full(pad, idx[-1], dtype=idx.dtype)])


def host_consts(w_qkv, pos_bias):
    import ml_dtypes

    bf16 = ml_dtypes.bfloat16
    f8 = ml_dtypes.float8_e4m3
    wq = w_qkv[0:512]
    wk = w_qkv[512:1024]
    wv = w_qkv[1024:1536]
    wqk8 = np.ascontiguousarray(
        np.clip(np.concatenate([wq, wk], axis=0).T * W_SCALE, -240, 240).astype(f8)
    )
    wvT = np.ascontiguousarray(wv.T.astype(bf16))
    # ebias[ab*64 + j, h*64 + i] = exp(pos_bias[h, i, j])
    big = np.zeros((64, 512), np.float32)
    for h in range(HEADS):
        big[:, h * 64 : (h + 1) * 64] = pos_bias[h].T
    ebiasT = np.ascontiguousarray(np.exp(np.tile(big, (2, 1))).astype(bf16))
    return wqk8, wvT, ebiasT


def kernel(x, w_qkv, pos_bias, focus_present_mask):
    global LAST_RESULT
    _ensure_ntff_hook()
    import ml_dtypes

    bf16 = ml_dtypes.bfloat16
    f8 = ml_dtypes.float8_e4m3

    x = np.ascontiguousarray(np.asarray(x), dtype=np.float32)
    w_qkv = np.asarray(w_qkv, dtype=np.float32)
    pos_bias = np.asarray(pos_bias, dtype=np.float32)
    mask = np.asarray(focus_present_mask).astype(bool)

    b, hw, n, dim = x.shape
    assert (n, dim) == (NTOK, DIM) and w_qkv.shape == (3 * HEADS * DH, DIM)
    x_flat = x.reshape(b * hw, n, dim)

    flat_idx = np.arange(b * hw)
    batch_of = flat_idx // hw
    attn_idx = flat_idx[~mask[batch_of]]
    vpr_idx = flat_idx[mask[batch_of]]

    # per-core granularity: 16 positions (one 1024-token block) x 8 cores
    attn_idx = _pad_positions(attn_idx, 16 * N_CORES) if len(attn_idx) else attn_idx
    vpr_idx = _pad_positions(vpr_idx, 16 * N_CORES) if len(vpr_idx) else vpr_idx
    a_pos_pc = len(attn_idx) // N_CORES
    v_pos_pc = len(vpr_idx) // N_CORES
    a_tok = a_pos_pc * NTOK
    v_tok = v_pos_pc * NTOK

    key = (a_tok, v_tok)
    if key not in _KERNEL_CACHE:
        _KERNEL_CACHE[key] = _build_kernel(a_tok, v_tok)
    nc = _KERNEL_CACHE[key]

    wqk8, wvT, ebiasT = host_consts(w_qkv, pos_bias)

    in_maps = []
    for core in range(N_CORES):
        m = {"wqk8": wqk8, "wvT": wvT, "ebiasT": ebiasT}
        if a_tok:
            ai = attn_idx[core * a_pos_pc : (core + 1) * a_pos_pc]
            xa = x_flat[ai].reshape(-1, DIM).T
            m["xa8T"] = np.ascontiguousarray(np.clip(xa, -240, 240).astype(f8))
            m["xaT"] = np.ascontiguousarray(xa.astype(bf16))
        if v_tok:
            vi = vpr_idx[core * v_pos_pc : (core + 1) * v_pos_pc]
            m["xvT"] = np.ascontiguousarray(x_flat[vi].reshape(-1, DIM).T.astype(bf16))
        in_maps.append(m)

    res = run_bass_kernel_spmd(nc, in_maps, core_ids=list(range(N_CORES)))
    LAST_RESULT = res

    out_flat = np.empty((b * hw, n, HEADS * DH), dtype=np.float32)
    for core in range(N_CORES):
        if a_tok:
            ai = attn_idx[core * a_pos_pc : (core + 1) * a_pos_pc]
            out_flat[ai] = (
                res.results[core]["out_a"]
                .astype(np.float32)
                .reshape(a_pos_pc, n, HEADS * DH)
            )
        if v_tok:
            vi = vpr_idx[core * v_pos_pc : (core + 1) * v_pos_pc]
            out_flat[vi] = (
                res.results[core]["out_v"]
                .astype(np.float32)
                .reshape(v_pos_pc, n, HEADS * DH)
            )
    return out_flat.reshape(b, hw, n, HEADS * DH)
